# revision 2
# baseline (speedup 1.0000x reference)
"""BSA kernel v4: everything on DVE in program order (no cross-engine
contention). S-tree per chunk (contiguous shift-by-8 adds on host-interleaved
sig) interleaved with chain chunks; A chunks DMA'd out as the chain finishes
them; masks computed host-side.
"""
import numpy as np

B, T, F = 8192, 2048, 20
NSTEPS = T - F                  # 2028
NCORES = 8
RPC = B // NCORES               # 1024
NG = RPC // 128                 # 8
THRESHOLD = 0.9952
CH = 256                        # chunk size (steps)
NCH = T // CH                   # 8 chunks
HALO = 34                       # tree halo in steps

_CACHE = {}


def _build_program(L):
    import concourse.bass as bass
    import concourse.mybir as mybir

    dt = mybir.dt.float32
    op = mybir.AluOpType

    nc = bass.Bass()
    sig_in = nc.declare_dram_parameter("sig_int", [128, T * NG], dt, isOutput=False)
    gneg_in = nc.declare_dram_parameter("gneg", [128, F * NG], dt, isOutput=False)
    out_d = nc.declare_dram_parameter("aout", [128, T * NG], dt, isOutput=True)

    ctxs = []

    def alloc(shape, dtype=dt):
        cm = nc.sbuf_tensor(shape, dtype)
        t = cm.__enter__()
        ctxs.append(cm)
        return t

    v = nc.vector

    W = T * NG
    SI = alloc([128, W])
    A = alloc([128, W])
    A3 = A[:, :].rearrange("p (t g) -> p t g", g=NG)
    tw = (CH + HALO) * NG + 64
    tA = alloc([128, tw])
    tB = alloc([128, tw])
    gneg = alloc([128, F * NG])
    gneg3 = gneg[:, :].rearrange("p (r g) -> p r g", g=NG)
    tmpA = alloc([128, F * NG])
    tmpB = alloc([128, F * NG])
    tA3 = tmpA[:, :].rearrange("p (r g) -> p r g", g=NG)
    tB3 = tmpB[:, :].rearrange("p (r g) -> p r g", g=NG)

    sem_sig = [nc.alloc_semaphore(f"sig_dma{c}") for c in range(NCH)]
    sem_g = nc.alloc_semaphore("gneg_load")
    sem_chain = nc.alloc_semaphore("chain")
    sem_out = nc.alloc_semaphore("out_dma")

    nc.sync.dma_start(out=gneg[:, :], in_=gneg_in[:, :]).then_inc(sem_g, 16)
    for c in range(NCH):
        lo = c * CH * NG
        hi = min(W, (c + 1) * CH * NG)
        nc.sync.dma_start(out=SI[:, lo:hi], in_=sig_in[:, lo:hi]).then_inc(
            sem_sig[c], 16)

    def tree(c):
        """S for chunk c (steps [c*CH, min((c+1)*CH, NSTEPS))) into A, on DVE."""
        lo_s = c * CH
        hi_s = min(T, (c + 1) * CH)
        ns = hi_s - lo_s
        n_out = max(0, min(hi_s, NSTEPS) - lo_s)
        if n_out == 0:
            return
        halo = min(HALO, T - hi_s)
        w_in = (ns + halo) * NG
        v.wait_ge(sem_sig[c], 16)
        if halo > 0 and hi_s < T:
            v.wait_ge(sem_sig[min(c + 1, NCH - 1)], 16)
        base = lo_s * NG
        s = SI[:, base:base + w_in]
        w1 = w_in - 1 * NG
        v.tensor_tensor(out=tA[:, 0:w1], in0=s[:, 0:w1], in1=s[:, NG:w1 + NG], op=op.add)
        w2 = w1 - 2 * NG
        v.tensor_tensor(out=tB[:, 0:w2], in0=tA[:, 0:w2], in1=tA[:, 2 * NG:w2 + 2 * NG], op=op.add)
        w3 = w2 - 4 * NG
        v.tensor_tensor(out=tA[:, 0:w3], in0=tB[:, 0:w3], in1=tB[:, 4 * NG:w3 + 4 * NG], op=op.add)
        w4 = w3 - 8 * NG
        v.tensor_tensor(out=tA[:, 0:w4], in0=tA[:, 0:w4], in1=tA[:, 8 * NG:w4 + 8 * NG], op=op.add)
        wout = n_out * NG
        v.tensor_tensor(out=A[:, base:base + wout], in0=tA[:, 0:wout],
                        in1=tB[:, 16 * NG:16 * NG + wout], op=op.add)

    v.wait_ge(sem_g, 16)
    tree(0)
    tree(1)

    nblk = (NSTEPS + CH - 1) // CH
    for t in range(NSTEPS):
        cur = A3[:, t:t + 1, :].broadcast_to([128, F, NG])
        tmp3 = tA3 if t % 2 == 0 else tB3
        v.scalar_tensor_tensor(
            out=tmp3[:, :, :], in0=cur, scalar=float(L), in1=gneg3[:, :, :],
            op0=op.is_ge, op1=op.mult,
        )
        fut = A3[:, t + 1:t + 1 + F, :]
        last = v.tensor_tensor(out=fut, in0=fut, in1=tmp3[:, :, :], op=op.add)
        if (t + 1) % CH == 0 or t == NSTEPS - 1:
            last.then_inc(sem_chain, 1)
            c_done = t // CH
            if c_done + 2 < NCH:
                tree(c_done + 2)

    for b in range(nblk):
        lo = b * CH * NG
        hi = min(W, (b + 1) * CH * NG)
        nc.sync.wait_ge(sem_chain, b + 1)
        nc.sync.dma_start(out=out_d[:, lo:hi], in_=A[:, lo:hi]).then_inc(
            sem_out, 16)
    nc.sync.wait_ge(sem_out, 16 * nblk)
    return nc


def kernel(sig: np.ndarray, filt: np.ndarray) -> np.ndarray:
    from concourse.bass_utils import run_bass_kernel_spmd

    sig = np.ascontiguousarray(np.asarray(sig, dtype=np.float32))
    filt = np.asarray(filt, dtype=np.float32)
    assert sig.shape == (B, T) and filt.shape == (F,)

    fsum = np.float32(filt.sum())
    L = np.float32(fsum / np.float32(1.0 + THRESHOLD))
    G = np.cumsum(filt[::-1].astype(np.float64))[::-1].astype(np.float32)

    key = (filt.tobytes(),)
    if _CACHE.get("key") != key:
        _CACHE["nc"] = _build_program(L)
        _CACHE["key"] = key
    nc = _CACHE["nc"]

    gneg = np.repeat(-G, NG).astype(np.float32)
    gneg = np.broadcast_to(gneg, (128, F * NG)).copy()

    in_maps = []
    for c in range(NCORES):
        blk = sig[c * RPC:(c + 1) * RPC]
        si = blk.reshape(NG, 128, T).transpose(1, 2, 0)
        si = np.ascontiguousarray(si.reshape(128, T * NG))
        in_maps.append({"sig_int": si, "gneg": gneg})

    res = run_bass_kernel_spmd(nc, in_maps, core_ids=list(range(NCORES)))

    out = np.empty((B, T), dtype=np.float32)
    for c in range(NCORES):
        a = res.results[c]["aout"].reshape(128, T, NG)
        m = (a[:, :NSTEPS, :] >= L)
        mc = np.zeros((NG, 128, T), dtype=np.float32)
        mc[:, :, :NSTEPS] = m.transpose(2, 0, 1)
        out[c * RPC:(c + 1) * RPC] = mc.reshape(RPC, T)
    return out


# revision 3
# speedup vs baseline: 1.0017x; 1.0017x over previous
"""BSA kernel v5: everything on DVE in program order (no cross-engine
contention). S-tree per chunk (contiguous shift-by-8 adds on host-interleaved
sig) interleaved with chain chunks; masks extracted on-device per chunk into
the dead sig staging region and DMA'd out as the chain advances. Host does
layout only (interleave in, de-interleave out).
"""
import numpy as np

B, T, F = 8192, 2048, 20
NSTEPS = T - F                  # 2028
NCORES = 8
RPC = B // NCORES               # 1024
NG = RPC // 128                 # 8
THRESHOLD = 0.9952
CH = 256                        # chunk size (steps)
NCH = T // CH                   # 8 chunks
HALO = 34                       # tree halo in steps

_CACHE = {}


def _build_program(L):
    import concourse.bass as bass
    import concourse.mybir as mybir

    dt = mybir.dt.float32
    op = mybir.AluOpType

    nc = bass.Bass()
    sig_in = nc.declare_dram_parameter("sig_int", [128, T * NG], dt, isOutput=False)
    gneg_in = nc.declare_dram_parameter("gneg", [128, F * NG], dt, isOutput=False)
    out_d = nc.declare_dram_parameter("mout", [128, T * NG], dt, isOutput=True)

    ctxs = []

    def alloc(shape, dtype=dt):
        cm = nc.sbuf_tensor(shape, dtype)
        t = cm.__enter__()
        ctxs.append(cm)
        return t

    v = nc.vector

    W = T * NG
    SI = alloc([128, W])
    A = alloc([128, W])
    A3 = A[:, :].rearrange("p (t g) -> p t g", g=NG)
    tw = (CH + HALO) * NG + 64
    tA = alloc([128, tw])
    tB = alloc([128, tw])
    gneg = alloc([128, F * NG])
    gneg3 = gneg[:, :].rearrange("p (r g) -> p r g", g=NG)
    tmpA = alloc([128, F * NG])
    tmpB = alloc([128, F * NG])
    tA3 = tmpA[:, :].rearrange("p (r g) -> p r g", g=NG)
    tB3 = tmpB[:, :].rearrange("p (r g) -> p r g", g=NG)

    sem_sig = [nc.alloc_semaphore(f"sig_dma{c}") for c in range(NCH)]
    sem_g = nc.alloc_semaphore("gneg_load")
    sem_chain = nc.alloc_semaphore("chain")
    sem_out = nc.alloc_semaphore("out_dma")

    nc.sync.dma_start(out=gneg[:, :], in_=gneg_in[:, :]).then_inc(sem_g, 16)
    for c in range(NCH):
        lo = c * CH * NG
        hi = min(W, (c + 1) * CH * NG)
        nc.sync.dma_start(out=SI[:, lo:hi], in_=sig_in[:, lo:hi]).then_inc(
            sem_sig[c], 16)

    def tree(c):
        """S for chunk c (steps [c*CH, min((c+1)*CH, NSTEPS))) into A, on DVE."""
        lo_s = c * CH
        hi_s = min(T, (c + 1) * CH)
        ns = hi_s - lo_s
        n_out = max(0, min(hi_s, NSTEPS) - lo_s)
        if n_out == 0:
            return
        halo = min(HALO, T - hi_s)
        w_in = (ns + halo) * NG
        v.wait_ge(sem_sig[c], 16)
        if halo > 0 and hi_s < T:
            v.wait_ge(sem_sig[min(c + 1, NCH - 1)], 16)
        base = lo_s * NG
        s = SI[:, base:base + w_in]
        w1 = w_in - 1 * NG
        v.tensor_tensor(out=tA[:, 0:w1], in0=s[:, 0:w1], in1=s[:, NG:w1 + NG], op=op.add)
        w2 = w1 - 2 * NG
        v.tensor_tensor(out=tB[:, 0:w2], in0=tA[:, 0:w2], in1=tA[:, 2 * NG:w2 + 2 * NG], op=op.add)
        w3 = w2 - 4 * NG
        v.tensor_tensor(out=tA[:, 0:w3], in0=tB[:, 0:w3], in1=tB[:, 4 * NG:w3 + 4 * NG], op=op.add)
        w4 = w3 - 8 * NG
        v.tensor_tensor(out=tA[:, 0:w4], in0=tA[:, 0:w4], in1=tA[:, 8 * NG:w4 + 8 * NG], op=op.add)
        wout = n_out * NG
        v.tensor_tensor(out=A[:, base:base + wout], in0=tA[:, 0:wout],
                        in1=tB[:, 16 * NG:16 * NG + wout], op=op.add)

    v.wait_ge(sem_g, 16)
    tree(0)
    tree(1)

    nblk = (NSTEPS + CH - 1) // CH
    for t in range(NSTEPS):
        cur = A3[:, t:t + 1, :].broadcast_to([128, F, NG])
        tmp3 = tA3 if t % 2 == 0 else tB3
        v.scalar_tensor_tensor(
            out=tmp3[:, :, :], in0=cur, scalar=float(L), in1=gneg3[:, :, :],
            op0=op.is_ge, op1=op.mult,
        )
        fut = A3[:, t + 1:t + 1 + F, :]
        last = v.tensor_tensor(out=fut, in0=fut, in1=tmp3[:, :, :], op=op.add)
        if (t + 1) % CH == 0 or t == NSTEPS - 1:
            c_done = t // CH
            # extract masks for chunk c_done into the dead SI region
            lo_e = c_done * CH * NG
            n_e = (min((c_done + 1) * CH, NSTEPS) - c_done * CH) * NG
            ext = v.tensor_scalar(
                out=SI[:, lo_e:lo_e + n_e], in0=A[:, lo_e:lo_e + n_e],
                scalar1=float(L), scalar2=None, op0=op.is_ge,
            )
            if t == NSTEPS - 1:
                ext = v.memset(SI[:, NSTEPS * NG:T * NG], 0.0)
            ext.then_inc(sem_chain, 1)
            if c_done + 2 < NCH:
                tree(c_done + 2)

    for b in range(nblk):
        lo = b * CH * NG
        hi = min(W, (b + 1) * CH * NG)
        nc.sync.wait_ge(sem_chain, b + 1)
        nc.sync.dma_start(out=out_d[:, lo:hi], in_=SI[:, lo:hi]).then_inc(
            sem_out, 16)
    nc.sync.wait_ge(sem_out, 16 * nblk)
    return nc


def kernel(sig: np.ndarray, filt: np.ndarray) -> np.ndarray:
    from concourse.bass_utils import run_bass_kernel_spmd

    sig = np.ascontiguousarray(np.asarray(sig, dtype=np.float32))
    filt = np.asarray(filt, dtype=np.float32)
    assert sig.shape == (B, T) and filt.shape == (F,)

    fsum = np.float32(filt.sum())
    L = np.float32(fsum / np.float32(1.0 + THRESHOLD))
    G = np.cumsum(filt[::-1].astype(np.float64))[::-1].astype(np.float32)

    key = (filt.tobytes(),)
    if _CACHE.get("key") != key:
        _CACHE["nc"] = _build_program(L)
        _CACHE["key"] = key
    nc = _CACHE["nc"]

    gneg = np.repeat(-G, NG).astype(np.float32)
    gneg = np.broadcast_to(gneg, (128, F * NG)).copy()

    in_maps = []
    for c in range(NCORES):
        blk = sig[c * RPC:(c + 1) * RPC]
        si = blk.reshape(NG, 128, T).transpose(1, 2, 0)
        si = np.ascontiguousarray(si.reshape(128, T * NG))
        in_maps.append({"sig_int": si, "gneg": gneg})

    res = run_bass_kernel_spmd(nc, in_maps, core_ids=list(range(NCORES)))

    out = np.empty((B, T), dtype=np.float32)
    for c in range(NCORES):
        m = res.results[c]["mout"].reshape(128, T, NG)
        out[c * RPC:(c + 1) * RPC] = np.ascontiguousarray(
            m.transpose(2, 0, 1).reshape(RPC, T))
    return out


# revision 4
# speedup vs baseline: 1.0061x; 1.0044x over previous
"""BSA kernel v5: everything on DVE in program order (no cross-engine
contention). S-tree per chunk (contiguous shift-by-8 adds on host-interleaved
sig) interleaved with chain chunks; masks extracted on-device per chunk into
the dead sig staging region and DMA'd out as the chain advances. Host does
layout only (interleave in, de-interleave out).
"""
import numpy as np

B, T, F = 8192, 2048, 20
NSTEPS = T - F                  # 2028
NCORES = 8
RPC = B // NCORES               # 1024
NG = RPC // 128                 # 8
THRESHOLD = 0.9952
CH = 256                        # chunk size (steps)
NCH = T // CH                   # 8 chunks
HALO = 34                       # tree halo in steps

_CACHE = {}


def _build_program(L):
    import concourse.bass as bass
    import concourse.mybir as mybir

    dt = mybir.dt.float32
    op = mybir.AluOpType

    nc = bass.Bass()
    sig_in = nc.declare_dram_parameter("sig_int", [128, T * NG], dt, isOutput=False)
    gneg_in = nc.declare_dram_parameter("gneg", [128, F * NG], dt, isOutput=False)
    out_d = nc.declare_dram_parameter("mout", [128, T * NG], dt, isOutput=True)

    ctxs = []

    def alloc(shape, dtype=dt):
        cm = nc.sbuf_tensor(shape, dtype)
        t = cm.__enter__()
        ctxs.append(cm)
        return t

    v = nc.vector

    W = T * NG
    SI = alloc([128, W])
    A = alloc([128, W])
    A3 = A[:, :].rearrange("p (t g) -> p t g", g=NG)
    tw = (CH + 64 + HALO) * NG + 64
    tA = alloc([128, tw])
    tB = alloc([128, tw])
    gneg = alloc([128, F * NG])
    gneg3 = gneg[:, :].rearrange("p (r g) -> p r g", g=NG)
    tmpA = alloc([128, F * NG])
    tmpB = alloc([128, F * NG])
    tA3 = tmpA[:, :].rearrange("p (r g) -> p r g", g=NG)
    tB3 = tmpB[:, :].rearrange("p (r g) -> p r g", g=NG)

    sem_sig = [nc.alloc_semaphore(f"sig_dma{c}") for c in range(NCH)]
    sem_g = nc.alloc_semaphore("gneg_load")
    sem_chain = nc.alloc_semaphore("chain")
    sem_out = nc.alloc_semaphore("out_dma")

    nc.sync.dma_start(out=gneg[:, :], in_=gneg_in[:, :]).then_inc(sem_g, 16)
    for c in range(NCH):
        lo = c * CH * NG
        hi = min(W, (c + 1) * CH * NG)
        nc.sync.dma_start(out=SI[:, lo:hi], in_=sig_in[:, lo:hi]).then_inc(
            sem_sig[c], 16)

    def tree_range(lo_s, hi_s):
        """S for steps [lo_s, min(hi_s, NSTEPS)) into A, on DVE."""
        ns = hi_s - lo_s
        n_out = max(0, min(hi_s, NSTEPS) - lo_s)
        if n_out == 0:
            return
        halo = min(HALO, T - hi_s)
        w_in = (ns + halo) * NG
        last_step = lo_s + (w_in // NG) - 1
        for cc in range(lo_s // CH, min(last_step // CH, NCH - 1) + 1):
            v.wait_ge(sem_sig[cc], 16)
        base = lo_s * NG
        s = SI[:, base:base + w_in]
        w1 = w_in - 1 * NG
        v.tensor_tensor(out=tA[:, 0:w1], in0=s[:, 0:w1], in1=s[:, NG:w1 + NG], op=op.add)
        w2 = w1 - 2 * NG
        v.tensor_tensor(out=tB[:, 0:w2], in0=tA[:, 0:w2], in1=tA[:, 2 * NG:w2 + 2 * NG], op=op.add)
        w3 = w2 - 4 * NG
        v.tensor_tensor(out=tA[:, 0:w3], in0=tB[:, 0:w3], in1=tB[:, 4 * NG:w3 + 4 * NG], op=op.add)
        w4 = w3 - 8 * NG
        v.tensor_tensor(out=tA[:, 0:w4], in0=tA[:, 0:w4], in1=tA[:, 8 * NG:w4 + 8 * NG], op=op.add)
        wout = n_out * NG
        v.tensor_tensor(out=A[:, base:base + wout], in0=tA[:, 0:wout],
                        in1=tB[:, 16 * NG:16 * NG + wout], op=op.add)

    v.wait_ge(sem_g, 16)
    tree_range(0, CH + 64)          # S for [0, 320): covers chain chunk 0 (+19)

    nblk = (NSTEPS + CH - 1) // CH
    for t in range(NSTEPS):
        cur = A3[:, t:t + 1, :].broadcast_to([128, F, NG])
        tmp3 = tA3 if t % 2 == 0 else tB3
        v.scalar_tensor_tensor(
            out=tmp3[:, :, :], in0=cur, scalar=float(L), in1=gneg3[:, :, :],
            op0=op.is_ge, op1=op.mult,
        )
        fut = A3[:, t + 1:t + 1 + F, :]
        last = v.tensor_tensor(out=fut, in0=fut, in1=tmp3[:, :, :], op=op.add)
        if (t + 1) % CH == 0 or t == NSTEPS - 1:
            c_done = t // CH
            # extract masks for chunk c_done into the dead SI region
            lo_e = c_done * CH * NG
            n_e = (min((c_done + 1) * CH, NSTEPS) - c_done * CH) * NG
            ext = v.tensor_scalar(
                out=SI[:, lo_e:lo_e + n_e], in0=A[:, lo_e:lo_e + n_e],
                scalar1=float(L), scalar2=None, op0=op.is_ge,
            )
            if t == NSTEPS - 1:
                ext = v.memset(SI[:, NSTEPS * NG:T * NG], 0.0)
            ext.then_inc(sem_chain, 1)
            if c_done == 0:
                tree_range(CH + 64, 2 * CH)   # S for [320, 512)
            if c_done + 2 < NCH:
                tree_range((c_done + 2) * CH, (c_done + 3) * CH)

    for b in range(nblk):
        lo = b * CH * NG
        hi = min(W, (b + 1) * CH * NG)
        nc.sync.wait_ge(sem_chain, b + 1)
        nc.sync.dma_start(out=out_d[:, lo:hi], in_=SI[:, lo:hi]).then_inc(
            sem_out, 16)
    nc.sync.wait_ge(sem_out, 16 * nblk)
    return nc


def kernel(sig: np.ndarray, filt: np.ndarray) -> np.ndarray:
    from concourse.bass_utils import run_bass_kernel_spmd

    sig = np.ascontiguousarray(np.asarray(sig, dtype=np.float32))
    filt = np.asarray(filt, dtype=np.float32)
    assert sig.shape == (B, T) and filt.shape == (F,)

    fsum = np.float32(filt.sum())
    L = np.float32(fsum / np.float32(1.0 + THRESHOLD))
    G = np.cumsum(filt[::-1].astype(np.float64))[::-1].astype(np.float32)

    key = (filt.tobytes(),)
    if _CACHE.get("key") != key:
        _CACHE["nc"] = _build_program(L)
        _CACHE["key"] = key
    nc = _CACHE["nc"]

    gneg = np.repeat(-G, NG).astype(np.float32)
    gneg = np.broadcast_to(gneg, (128, F * NG)).copy()

    in_maps = []
    for c in range(NCORES):
        blk = sig[c * RPC:(c + 1) * RPC]
        si = blk.reshape(NG, 128, T).transpose(1, 2, 0)
        si = np.ascontiguousarray(si.reshape(128, T * NG))
        in_maps.append({"sig_int": si, "gneg": gneg})

    res = run_bass_kernel_spmd(nc, in_maps, core_ids=list(range(NCORES)))

    out = np.empty((B, T), dtype=np.float32)
    for c in range(NCORES):
        m = res.results[c]["mout"].reshape(128, T, NG)
        out[c * RPC:(c + 1) * RPC] = np.ascontiguousarray(
            m.transpose(2, 0, 1).reshape(RPC, T))
    return out
